# revision 4
# baseline (speedup 1.0000x reference)
"""Trainium2 Bass kernel for nn_Actformer (scatter_memory).

Math (per batch b):
  q = wq @ query[b] + bq                   # [64]     (host)
  M[h,:] = 0.25 * Wk_h^T q_h               # [4,64]   (host; collapses K-proj, bk
                                           #           cancels in softmax)
  scores[h,s] = M[h,:] . sp[b,s,:]         # [4,S]    (device, fp8 matmul, x16 scale)
  attn = softmax_s(scores)
  u[h,:] = sum_s attn[h,s] sp[b,s,:]       # [4,64]   (collapses V-projection)
  value = sum_h (wo_h @ wv_h) u_h          # [64]     (Wov precomputed on host)
  h1 = relu(w_a1 @ value + b_a1)
  a = softmax(w_a2 @ h1 + b_a2)            # [S]      (b_a2 zeros by spec)
  w = w_write @ value + b_write
  out[b,s,:] = sp[b,s,:] + a[s]*(w - sp[b,s,:])

Sharding: pure data parallel, batch 1024 -> 128 per core across 8 cores.

Device layouts (per core, 4-batch DMA blocks for ~1MB transfers):
  nat4 [32, 128, 4160] bf16: per batch i in block: cols [i*1040, i*1040+1024) hold
      sp in (d,n) order (col d*16+n = sp[s=n*128+q, d], partition q); cols
      [i*1040+1024, i*1040+1040) hold ones (softmax denominator via matmul).
  t2f8 [32, 128, 4128] fp8e4m3: per batch i: cols [i*1032, i*1032+1024) hold spT2
      (partition 2d+c, col j*128+q, s = j*256+c*128+q); cols [+1024, +1032) hold
      16*M in the same partition space (even partitions head cols 0-3, odd 4-7).
  out4 [32, 128, 4096] bf16: same (d,n) order as nat4 minus the ones columns.

Attention: scores via 8 fp8 matmuls (lhsT = spT2 chunk stationary, rhs = md),
exp with scale 1/16 on ACT, u via 16 accumulating matmuls against the native
layout (ones column gives the softmax denominator for free).

Address net (transposed): logitsT[q,(k,bl)] via 16 matmuls lhsT=wa2T chunk,
rhs=h1; one exp [128,256]; denominator by ones-column matmul + strided reduce;
normalize+reorder to a2T[q,(bl,k)] in one DVE op. No DRAM round trip, no PE
transposes of the address vector.

Update per batch (native layout, full-width [128,1024] ops):
  delta = w_bcast - sp      (gpsimd TT, w broadcast along free via stride-0 AP)
  p     = delta * a_rep     (DVE TT 2x mode, a_rep innermost-contiguous)
  out   = sp + p            (DVE TT 2x mode)
w_bcast [128,64] comes from a per-batch select matmul (sel one-hot lhsT x WT).
"""

import numpy as np
import ml_dtypes

import bass_rust
import concourse.bass as bass


def _install_ntff_hook():
    """The agent image lacks antenv.axon_hooks, so run_bass_kernel_spmd's
    trace path degrades. Recreate the hook (ctypes into libaxon_pjrt.so)
    and inject it as the antenv.axon_hooks module."""
    import sys
    import types
    import ctypes
    import contextlib

    if "antenv.axon_hooks" in sys.modules:
        return
    so_path = "/opt/axon/libaxon_pjrt.so"
    try:
        lib = ctypes.CDLL(so_path)
    except OSError:
        return
    if not hasattr(lib, "axon_start_nrt_profile"):
        return
    lib.axon_start_nrt_profile.argtypes = [
        ctypes.POINTER(ctypes.c_int64),
        ctypes.c_size_t,
    ]
    lib.axon_start_nrt_profile.restype = ctypes.c_int64
    lib.axon_stop_nrt_profile.argtypes = [ctypes.c_char_p]
    lib.axon_stop_nrt_profile.restype = ctypes.c_int64

    @contextlib.contextmanager
    def _hook(output_dir, device_ids):
        import jax

        jax.devices()
        if device_ids:
            ids = (ctypes.c_int64 * len(device_ids))(*device_ids)
            rc = lib.axon_start_nrt_profile(ids, len(device_ids))
        else:
            rc = lib.axon_start_nrt_profile(None, 0)
        if rc != 0:
            raise RuntimeError(f"axon_start_nrt_profile rc={rc}")
        try:
            yield
        finally:
            n = lib.axon_stop_nrt_profile(str(output_dir).encode())
            print(f"profile: {n} file(s) written to {output_dir}")

    mod = types.ModuleType("antenv.axon_hooks")
    mod.get_axon_ntff_profile_hook = lambda: _hook
    mod.set_axon_ntff_profile_hook = lambda h: None
    sys.modules["antenv.axon_hooks"] = mod


_install_ntff_hook()
import concourse.mybir as mybir
from concourse.masks import make_identity
from concourse.tile import TileContext
from concourse.bass_utils import run_bass_kernel_spmd

B, S, D, H, HD = 1024, 2048, 64, 4, 16
NCORES = 8
BL = B // NCORES          # 128 batches per core
G = 16                    # batches per group (address-net batching)
NB4 = BL // 4             # 4-batch DMA blocks
NCH = 16                  # s-chunks per batch (s = n*128 + q)
NATW = 1040               # per-batch native cols: 1024 sp + 16 ones
T2W = 1032                # per-batch t2 cols: 1024 spT2 + 8 md

BF16 = mybir.dt.bfloat16
F32 = mybir.dt.float32
F8 = mybir.dt.float8e4
AF = mybir.ActivationFunctionType
ALU = mybir.AluOpType

_CACHE = {}


def _sap(tile_ap, col0, ap_rest):
    """Strided view of an SBUF tile: keep the partition dim, replace free dims."""
    base = tile_ap[:, col0 : col0 + 1]
    return bass.AP(
        tensor=base.tensor,
        offset=base.offset,
        ap=[list(base.ap[0])] + [list(x) for x in ap_rest],
    )


def _build():
    if "nc" in _CACHE:
        return _CACHE["nc"]
    nc = bass.Bass()

    nat4 = nc.dram_tensor("nat4", [NB4, 128, 4 * NATW], BF16, kind="ExternalInput")
    t2f8 = nc.dram_tensor("t2f8", [NB4, 128, 4 * T2W], F8, kind="ExternalInput")
    wovT = nc.dram_tensor("wovT", [64, 256], BF16, kind="ExternalInput")
    wa1T65 = nc.dram_tensor("wa1T65", [65, 128], BF16, kind="ExternalInput")
    wa2T = nc.dram_tensor("wa2T", [128, 2048], BF16, kind="ExternalInput")
    wwT65 = nc.dram_tensor("wwT65", [65, 64], BF16, kind="ExternalInput")
    sel = nc.dram_tensor("sel", [16, 2048], BF16, kind="ExternalInput")
    out4 = nc.dram_tensor("out4", [NB4, 128, 4096], BF16, kind="ExternalOutput")

    with TileContext(nc) as tc:
        with (
            tc.tile_pool(name="const", bufs=1) as const,
            tc.tile_pool(name="natp", bufs=8) as natp,
            tc.tile_pool(name="t2p", bufs=4) as t2p,
            tc.tile_pool(name="ep", bufs=6) as ep,
            tc.tile_pool(name="grp", bufs=2) as grp,
            tc.tile_pool(name="dp", bufs=4) as dp,
            tc.tile_pool(name="pp", bufs=4) as pp,
            tc.tile_pool(name="wbp", bufs=4) as wbp,
            tc.tile_pool(name="outp", bufs=4) as outp,
            tc.tile_pool(name="ps_sc", bufs=2, space="PSUM") as ps_sc,
            tc.tile_pool(name="ps_u", bufs=2, space="PSUM") as ps_u,
            tc.tile_pool(name="ps_wb", bufs=2, space="PSUM") as ps_wb,
            tc.tile_pool(name="ps_g", bufs=2, space="PSUM") as ps_g,
        ):
            # ---- constants ----
            wovT_sb = const.tile([64, 256], BF16)
            nc.sync.dma_start(out=wovT_sb, in_=wovT[:, :])
            wa1T_sb = const.tile([65, 128], BF16)
            nc.sync.dma_start(out=wa1T_sb, in_=wa1T65[:, :])
            wa2T_sb = const.tile([128, 2048], BF16)
            nc.sync.dma_start(out=wa2T_sb, in_=wa2T[:, :])
            wwT_sb = const.tile([65, 64], BF16)
            nc.sync.dma_start(out=wwT_sb, in_=wwT65[:, :])
            sel_sb = const.tile([16, 2048], BF16)
            nc.sync.dma_start(out=sel_sb, in_=sel[:, :])
            ident4 = const.tile([4, 4], BF16)
            make_identity(nc, ident4[:, :])
            ones_col = const.tile([128, 1], BF16)
            nc.gpsimd.memset(ones_col[:, :], 1.0)
            ones_row = const.tile([1, 128], BF16)
            nc.gpsimd.memset(ones_row[:, :], 1.0)

            nat_tiles = {}
            t2_tiles = {}

            for g in range(BL // G):
                # ================= phase A: per-batch attention =================
                utg = grp.tile([64, 4 * G], BF16, tag="utg")
                for bl in range(G):
                    b = g * G + bl
                    t, i = b // 4, b % 4
                    if i == 0:
                        nt = natp.tile([128, 4 * NATW], BF16, tag="nat")
                        nc.sync.dma_start(out=nt, in_=nat4[t, :, :])
                        t2t = t2p.tile([128, 4 * T2W], F8, tag="t2")
                        nc.scalar.dma_start(out=t2t, in_=t2f8[t, :, :])
                        nat_tiles[t] = nt
                        t2_tiles[t] = t2t
                    nt = nat_tiles[t]
                    t2t = t2_tiles[t]
                    c0n = i * NATW
                    c0t = i * T2W

                    # scores: 8 fp8 matmuls -> [128, 64] PSUM (16x scale)
                    sc_ps = ps_sc.tile([128, 64], F32, tag="sc")
                    md = t2t[:, c0t + 1024 : c0t + 1032]
                    for j in range(8):
                        nc.tensor.matmul(
                            sc_ps[:, j * 8 : (j + 1) * 8],
                            t2t[:, c0t + j * 128 : c0t + (j + 1) * 128],
                            md,
                            start=True,
                            stop=True,
                        )
                    e_sb = ep.tile([128, 64], BF16, tag="esc")
                    nc.scalar.activation(out=e_sb, in_=sc_ps, func=AF.Exp, scale=0.0625)

                    # u = sum_s exp_score * sp; strided rhs picks chunk n plus
                    # its ones column (offset n + 16*64 = n + 1024).
                    u_ps = ps_u.tile([4, 65], F32, tag="ub")
                    for n in range(NCH):
                        nc.tensor.matmul(
                            u_ps,
                            e_sb[:, 4 * n : 4 * n + 4],
                            _sap(nt, c0n + n, [[16, 65]]),
                            start=(n == 0),
                            stop=(n == NCH - 1),
                        )
                    inv_sb = ep.tile([4, 1], F32, tag="inv")
                    nc.vector.reciprocal(inv_sb, u_ps[:, 64:65])
                    u_sb = ep.tile([4, 64], BF16, tag="usb")
                    nc.vector.tensor_scalar_mul(u_sb, u_ps[:, 0:64], inv_sb)
                    ut_ps = ps_u.tile([64, 4], BF16, tag="ub")
                    nc.tensor.transpose(ut_ps, u_sb, ident4)
                    nc.scalar.activation(
                        out=utg[:, bl * 4 : (bl + 1) * 4], in_=ut_ps, func=AF.Copy
                    )

                # ================= phase B: group value + address =================
                utg_v = utg[:].rearrange("p (b h) -> p h b", h=4)
                v_ps = ps_g.tile([64, G], F32, tag="g")
                for h in range(4):
                    nc.tensor.matmul(
                        v_ps,
                        wovT_sb[:, h * 64 : (h + 1) * 64],
                        utg_v[:, h : h + 1, :],
                        start=(h == 0),
                        stop=(h == 3),
                    )
                v65 = grp.tile([65, G], BF16, tag="v65")
                nc.scalar.activation(out=v65[0:64, :], in_=v_ps, func=AF.Copy)
                nc.vector.memset(v65[64:65, :], 1.0)

                h1_ps = ps_g.tile([128, G], F32, tag="g")
                nc.tensor.matmul(h1_ps, wa1T_sb, v65, start=True, stop=True)
                h1_sb = grp.tile([128, G], BF16, tag="h1")
                nc.vector.tensor_scalar_max(h1_sb, h1_ps, 0.0)

                wt_ps = ps_g.tile([16, 64], F32, tag="g")
                nc.tensor.matmul(wt_ps, v65, wwT_sb, start=True, stop=True)
                wt_sb = grp.tile([16, 64], BF16, tag="wt")
                nc.scalar.activation(out=wt_sb, in_=wt_ps, func=AF.Copy)

                # logitsT[q, k*16+bl]
                lg_ps = ps_g.tile([128, 16 * G], F32, tag="g")
                for k in range(16):
                    nc.tensor.matmul(
                        lg_ps[:, k * G : (k + 1) * G],
                        wa2T_sb[:, k * 128 : (k + 1) * 128],
                        h1_sb,
                        start=True,
                        stop=True,
                    )
                e2T = grp.tile([128, 16 * G], BF16, tag="e2T")
                nc.scalar.activation(out=e2T, in_=lg_ps, func=AF.Exp)

                # denominators: column-sum via ones matmul, then strided reduce
                ds_ps = ps_g.tile([1, 16 * G], F32, tag="g")
                nc.tensor.matmul(ds_ps, ones_col, e2T, start=True, stop=True)
                den_sb = grp.tile([1, G], F32, tag="den")
                nc.vector.reduce_sum(
                    out=den_sb,
                    in_=_sap(ds_ps, 0, [[1, G], [G, 16]]),
                    axis=mybir.AxisListType.X,
                )
                linv_sb = grp.tile([1, G], BF16, tag="linv")
                with nc.allow_low_precision(
                    reason="softmax 1/denom in bf16; scales a tiny update term"
                ):
                    nc.vector.reciprocal(linv_sb, den_sb)
                lb_ps = ps_g.tile([128, G], F32, tag="g")
                nc.tensor.matmul(lb_ps, ones_row, linv_sb, start=True, stop=True)

                # a2T[q, bl*16+k] = e2T[q, k*16+bl] * linv[bl]
                a2T = grp.tile([128, 16 * G], BF16, tag="a2T")
                nc.vector.tensor_tensor(
                    out=_sap(a2T, 0, [[G, 16], [1, 16]]),
                    in0=_sap(e2T, 0, [[1, 16], [G, 16]]),
                    in1=_sap(lb_ps, 0, [[1, 16], [0, 16]]),
                    op=ALU.mult,
                )

                # ================= phase C: per-batch scatter update =================
                for bl in range(G):
                    b = g * G + bl
                    t, i = b // 4, b % 4
                    nt = nat_tiles[t]
                    c0n = i * NATW
                    if i == 0:
                        ot = outp.tile([128, 4096], BF16, tag="ob")
                        nat_tiles[(t, "out")] = ot
                    ot = nat_tiles[(t, "out")]

                    wb_ps = ps_wb.tile([128, 64], F32, tag="wb")
                    nc.tensor.matmul(
                        wb_ps,
                        sel_sb[:, bl * 128 : (bl + 1) * 128],
                        wt_sb,
                        start=True,
                        stop=True,
                    )
                    wb_sb = wbp.tile([128, 64], BF16, tag="wbs")
                    nc.scalar.activation(out=wb_sb, in_=wb_ps, func=AF.Copy)

                    sp2d = _sap(nt, c0n, [[16, 64], [1, 16]])
                    dt_ = dp.tile([128, 1024], BF16, tag="dt")
                    nc.gpsimd.tensor_tensor(
                        out=_sap(dt_, 0, [[16, 64], [1, 16]]),
                        in0=_sap(wb_sb, 0, [[1, 64], [0, 16]]),
                        in1=sp2d,
                        op=ALU.subtract,
                    )
                    pt = pp.tile([128, 1024], BF16, tag="pt")
                    nc.vector.tensor_tensor(
                        out=_sap(pt, 0, [[16, 64], [1, 16]]),
                        in0=_sap(dt_, 0, [[16, 64], [1, 16]]),
                        in1=_sap(a2T, bl * 16, [[0, 64], [1, 16]]),
                        op=ALU.mult,
                    )
                    nc.vector.tensor_tensor(
                        out=ot[:, i * 1024 : (i + 1) * 1024],
                        in0=nt[:, c0n : c0n + 1024],
                        in1=pt[:, :],
                        op=ALU.add,
                    )
                    if i == 3:
                        nc.sync.dma_start(out=out4[t, :, :], in_=ot)

    _split_dma_waits(nc)
    _CACHE["nc"] = nc
    return nc


def _split_dma_waits(nc):
    """walrus's DMA pseudo-instruction encodes at most one sem wait; move
    extra waits emitted by Tile onto a NoOp right before the DMA."""
    k = 0
    for f in nc.m.functions:
        for blk in f.blocks:
            insts = list(blk.instructions)
            new = []
            changed = False
            for inst in insts:
                si = inst.sync_info
                if si is not None and len(si.on_wait) > 1:
                    waits = list(si.on_wait)
                    for w in waits[:-1]:
                        nop = mybir.InstNoOp(name=f"WSPLIT-{k}", ins=[], outs=[])
                        k += 1
                        nop.engine = inst.engine
                        nop.sync_info = bass_rust.SyncInfo(
                            on_wait=[w], on_update=[]
                        )
                        new.append(nop)
                    inst.sync_info = bass_rust.SyncInfo(
                        on_wait=[waits[-1]], on_update=list(si.on_update)
                    )
                    changed = True
                new.append(inst)
            if changed:
                blk.instructions = new


def _host_prep(query, scratchpad, wq, wk, wv, bq, wo, w_a1, b_a1, w_a2,
               w_write, b_write):
    """Build per-core input maps (numpy, all host-side)."""
    bf = ml_dtypes.bfloat16
    f8 = ml_dtypes.float8_e4m3
    # query-side collapse: M[b,h,:] = 0.25 * Wk_h^T q_h, scaled x16 for fp8
    q = query[:, 0, :] @ wq.T + bq                         # [B, 64]
    M = 4.0 * np.einsum(
        "hjd,bhj->bhd", wk.reshape(H, HD, D), q.reshape(B, H, HD)
    )                                                      # [B, H, 64], 16*0.25
    mdup = np.zeros((B, 128, 8), np.float32)
    mt = M.transpose(0, 2, 1)                              # [B, 64, H]
    mdup[:, 0::2, 0:4] = mt                                # T2 partition = 2d+c
    mdup[:, 1::2, 4:8] = mt

    # t2 blocks: spT2 + md, fp8, 4 batches per row
    spT2 = np.ascontiguousarray(
        scratchpad.reshape(B, 8, 2, 128, 64)
        .transpose(0, 4, 2, 1, 3)     # [b, d, c, j, q] -> partition 2d+c
        .reshape(B, 128, 1024)
    )
    t2 = np.concatenate([spT2, mdup], axis=2).astype(f8)   # [B, 128, 1032]
    t2 = (
        t2.reshape(B // 4, 4, 128, T2W)
        .transpose(0, 2, 1, 3)
        .reshape(B // 4, 128, 4 * T2W)
    )

    # native blocks: (d,n) layout + ones cols, bf16, 4 batches per row
    spn = (
        scratchpad.reshape(B, NCH, 128, 64)
        .transpose(0, 2, 3, 1)        # [b, q, d, n]
        .reshape(B, 128, 1024)
    )
    nat = np.empty((B, 128, NATW), np.float32)
    nat[:, :, 0:1024] = spn
    nat[:, :, 1024:NATW] = 1.0
    nat = nat.astype(bf)
    nat = (
        nat.reshape(B // 4, 4, 128, NATW)
        .transpose(0, 2, 1, 3)
        .reshape(B // 4, 128, 4 * NATW)
    )

    wovT = np.concatenate(
        [(wo[:, h * HD : (h + 1) * HD] @ wv[h * HD : (h + 1) * HD, :]).T
         for h in range(H)],
        axis=1,
    )                                                      # [64, 256]
    wa1T65 = np.concatenate([w_a1.T, b_a1[None, :]], axis=0)   # [65, 128]
    wa2T = np.ascontiguousarray(w_a2.T)                    # [128, 2048]
    wwT65 = np.concatenate([w_write.T, b_write[None, :]], axis=0)  # [65, 64]
    sel = np.zeros((16, 2048), np.float32)
    for p in range(16):
        sel[p, p * 128 : (p + 1) * 128] = 1.0

    shared = {
        "wovT": wovT.astype(bf),
        "wa1T65": np.ascontiguousarray(wa1T65).astype(bf),
        "wa2T": wa2T.astype(bf),
        "wwT65": np.ascontiguousarray(wwT65).astype(bf),
        "sel": sel.astype(bf),
    }
    in_maps = []
    nblk = NB4
    for c in range(NCORES):
        lo, hi = c * nblk, (c + 1) * nblk
        in_maps.append(
            {"nat4": nat[lo:hi], "t2f8": t2[lo:hi], **shared}
        )
    return in_maps


def run(inputs, trace=False, **trace_kwargs):
    nc = _build()
    in_maps = _host_prep(
        np.asarray(inputs["query"], np.float32),
        np.asarray(inputs["scratchpad"], np.float32),
        np.asarray(inputs["wq"], np.float32),
        np.asarray(inputs["wk"], np.float32),
        np.asarray(inputs["wv"], np.float32),
        np.asarray(inputs["bq"], np.float32),
        np.asarray(inputs["wo"], np.float32),
        np.asarray(inputs["w_a1"], np.float32),
        np.asarray(inputs["b_a1"], np.float32),
        np.asarray(inputs["w_a2"], np.float32),
        np.asarray(inputs["w_write"], np.float32),
        np.asarray(inputs["b_write"], np.float32),
    )
    res = run_bass_kernel_spmd(
        nc, in_maps, core_ids=list(range(NCORES)), trace=trace, **trace_kwargs
    )
    outs = []
    for c in range(NCORES):
        o = np.asarray(res.results[c]["out4"]).astype(np.float32)
        outs.append(
            o.reshape(NB4, 128, 4, 64, 16)
            .transpose(0, 2, 4, 1, 3)     # [t, i, n, q, d]
            .reshape(BL, S, D)
        )
    full = np.concatenate(outs, axis=0)
    return full, res


def kernel(**inputs):
    full, _ = run(inputs, trace=False)
    return full


# revision 6
# speedup vs baseline: 1.1828x; 1.1828x over previous
"""Trainium2 Bass kernel for nn_Actformer (scatter_memory).

Math (per batch b):
  q = wq @ query[b] + bq                   # [64]     (host)
  M[h,:] = 0.25 * Wk_h^T q_h               # [4,64]   (host; collapses K-proj, bk
                                           #           cancels in softmax)
  scores[h,s] = M[h,:] . sp[b,s,:]         # [4,S]    (device, fp8 matmul, x16 scale)
  attn = softmax_s(scores)
  u[h,:] = sum_s attn[h,s] sp[b,s,:]       # [4,64]   (collapses V-projection)
  value = sum_h (wo_h @ wv_h) u_h          # [64]     (Wov precomputed on host)
  h1 = relu(w_a1 @ value + b_a1)
  a = softmax(w_a2 @ h1 + b_a2)            # [S]      (b_a2 zeros by spec)
  w = w_write @ value + b_write
  out[b,s,:] = sp[b,s,:] + a[s]*(w - sp[b,s,:])

Sharding: pure data parallel, batch 1024 -> 128 per core across 8 cores.

Device layouts (per core, 4-batch DMA blocks for ~1MB transfers):
  nat4 [32, 128, 4096] bf16: per batch i in block: cols [i*1024, (i+1)*1024) =
      sp in (d,n) order (col d*16+n = sp[s=n*128+q, d], partition q).
  t2f8 [32, 128, 4128] fp8e4m3: per batch i: cols [i*1032, i*1032+1024) = spT2
      (partition 2d+c, col j*128+q, s = j*256+c*128+q); cols [+1024, +1032) =
      16*M in the same partition space (even partitions head cols 0-3, odd 4-7).
  out4 [32, 128, 4096] bf16: same (d,n) order as nat4.

Attention: scores via 8 fp8 matmuls (lhsT = spT2 chunk stationary, rhs = md),
exp with scale 1/16 on ACT; u via two interleaved accumulation chains (even/odd
chunks) to avoid PSUM accumulate-chain serialization; per-head softmax
denominator via two small matmuls (e x ones -> [64,1]; H4 x that -> [4,1]).

Address net (transposed): logitsT[q,(k,bl)] via 16 matmuls lhsT=wa2T chunk,
rhs=h1; one exp [128,256]; denominator by ones-column matmul + strided reduce;
normalize+reorder to a2T[q,(bl,k)] in one DVE op.

Update per 4-batch block, [128,4096] ops (DVE-perf-mode-aware: dense two-read
pairs go to scalar_tensor_tensor, broadcasts keep innermost stride 0):
  d4 = wb4_rep - nat4       (TT, broadcast in0; split DVE/gpsimd for balance)
  p4 = d4 * a_rep           (DVE TT, broadcast in1)
  o4 = (nat4 * 1) + p4      (DVE STT, dense)
wb4 [128,4*64] from four select matmuls (one-hot sel lhsT x WT) + one copy.
"""

import numpy as np
import ml_dtypes

import bass_rust
import concourse.bass as bass


def _install_ntff_hook():
    """The agent image lacks antenv.axon_hooks, so run_bass_kernel_spmd's
    trace path degrades. Recreate the hook (ctypes into libaxon_pjrt.so)
    and inject it as the antenv.axon_hooks module."""
    import sys
    import types
    import ctypes
    import contextlib

    if "antenv.axon_hooks" in sys.modules:
        return
    so_path = "/opt/axon/libaxon_pjrt.so"
    try:
        lib = ctypes.CDLL(so_path)
    except OSError:
        return
    if not hasattr(lib, "axon_start_nrt_profile"):
        return
    lib.axon_start_nrt_profile.argtypes = [
        ctypes.POINTER(ctypes.c_int64),
        ctypes.c_size_t,
    ]
    lib.axon_start_nrt_profile.restype = ctypes.c_int64
    lib.axon_stop_nrt_profile.argtypes = [ctypes.c_char_p]
    lib.axon_stop_nrt_profile.restype = ctypes.c_int64

    @contextlib.contextmanager
    def _hook(output_dir, device_ids):
        import jax

        jax.devices()
        if device_ids:
            ids = (ctypes.c_int64 * len(device_ids))(*device_ids)
            rc = lib.axon_start_nrt_profile(ids, len(device_ids))
        else:
            rc = lib.axon_start_nrt_profile(None, 0)
        if rc != 0:
            raise RuntimeError(f"axon_start_nrt_profile rc={rc}")
        try:
            yield
        finally:
            n = lib.axon_stop_nrt_profile(str(output_dir).encode())
            print(f"profile: {n} file(s) written to {output_dir}")

    mod = types.ModuleType("antenv.axon_hooks")
    mod.get_axon_ntff_profile_hook = lambda: _hook
    mod.set_axon_ntff_profile_hook = lambda h: None
    sys.modules["antenv.axon_hooks"] = mod


_install_ntff_hook()
import concourse.mybir as mybir
from concourse.masks import make_identity
from concourse.tile import TileContext
from concourse.bass_utils import run_bass_kernel_spmd

B, S, D, H, HD = 1024, 2048, 64, 4, 16
NCORES = 8
BL = B // NCORES          # 128 batches per core
G = 16                    # batches per group (address-net batching)
NB4 = BL // 4             # 4-batch DMA blocks
NCH = 16                  # s-chunks per batch (s = n*128 + q)
T2W = 1032                # per-batch t2 cols: 1024 spT2 + 8 md

BF16 = mybir.dt.bfloat16
F32 = mybir.dt.float32
F8 = mybir.dt.float8e4
AF = mybir.ActivationFunctionType
ALU = mybir.AluOpType

_CACHE = {}


def _sap(tile_ap, col0, ap_rest):
    """Strided view of an SBUF/PSUM tile: keep the partition dim, replace
    free dims with explicit [stride, count] pairs."""
    base = tile_ap[:, col0 : col0 + 1]
    return bass.AP(
        tensor=base.tensor,
        offset=base.offset,
        ap=[list(base.ap[0])] + [list(x) for x in ap_rest],
    )


def _build():
    if "nc" in _CACHE:
        return _CACHE["nc"]
    nc = bass.Bass()

    nat4 = nc.dram_tensor("nat4", [NB4, 128, 4096], BF16, kind="ExternalInput")
    t2f8 = nc.dram_tensor("t2f8", [NB4, 128, 4 * T2W], F8, kind="ExternalInput")
    wovT = nc.dram_tensor("wovT", [64, 256], BF16, kind="ExternalInput")
    wa1T65 = nc.dram_tensor("wa1T65", [65, 128], BF16, kind="ExternalInput")
    wa2T = nc.dram_tensor("wa2T", [128, 2048], BF16, kind="ExternalInput")
    wwT65 = nc.dram_tensor("wwT65", [65, 64], BF16, kind="ExternalInput")
    sel = nc.dram_tensor("sel", [16, 2048], BF16, kind="ExternalInput")
    h4 = nc.dram_tensor("h4", [64, 4], BF16, kind="ExternalInput")
    out4 = nc.dram_tensor("out4", [NB4, 128, 4096], BF16, kind="ExternalOutput")

    with TileContext(nc) as tc:
        with (
            tc.tile_pool(name="const", bufs=1) as const,
            tc.tile_pool(name="natp", bufs=8) as natp,
            tc.tile_pool(name="t2p", bufs=4) as t2p,
            tc.tile_pool(name="ep", bufs=4) as ep,
            tc.tile_pool(name="grp", bufs=2) as grp,
            tc.tile_pool(name="dp", bufs=3) as dp,
            tc.tile_pool(name="pp", bufs=3) as pp,
            tc.tile_pool(name="wbp", bufs=3) as wbp,
            tc.tile_pool(name="outp", bufs=3) as outp,
            tc.tile_pool(name="ps_sc", bufs=2, space="PSUM") as ps_sc,
            tc.tile_pool(name="ps_u", bufs=2, space="PSUM") as ps_u,
            tc.tile_pool(name="ps_wb", bufs=2, space="PSUM") as ps_wb,
            tc.tile_pool(name="ps_g", bufs=2, space="PSUM") as ps_g,
        ):
            # ---- constants ----
            wovT_sb = const.tile([64, 256], BF16)
            nc.sync.dma_start(out=wovT_sb, in_=wovT[:, :])
            wa1T_sb = const.tile([65, 128], BF16)
            nc.sync.dma_start(out=wa1T_sb, in_=wa1T65[:, :])
            wa2T_sb = const.tile([128, 2048], BF16)
            nc.sync.dma_start(out=wa2T_sb, in_=wa2T[:, :])
            wwT_sb = const.tile([65, 64], BF16)
            nc.sync.dma_start(out=wwT_sb, in_=wwT65[:, :])
            sel_sb = const.tile([16, 2048], BF16)
            nc.sync.dma_start(out=sel_sb, in_=sel[:, :])
            h4_sb = const.tile([64, 4], BF16)
            nc.sync.dma_start(out=h4_sb, in_=h4[:, :])
            ident4 = const.tile([4, 4], BF16)
            make_identity(nc, ident4[:, :])
            ones_col = const.tile([128, 1], BF16)
            nc.gpsimd.memset(ones_col[:, :], 1.0)
            ones_row = const.tile([1, 128], BF16)
            nc.gpsimd.memset(ones_row[:, :], 1.0)

            nat_tiles = {}
            t2_tiles = {}

            for g in range(BL // G):
                # ================= phase A: per-batch attention =================
                utg = grp.tile([64, 4 * G], BF16, tag="utg")
                for bl in range(G):
                    b = g * G + bl
                    t, i = b // 4, b % 4
                    if i == 0:
                        nt = natp.tile([128, 4096], BF16, tag="nat")
                        nc.sync.dma_start(out=nt, in_=nat4[t, :, :])
                        t2t = t2p.tile([128, 4 * T2W], F8, tag="t2")
                        nc.scalar.dma_start(out=t2t, in_=t2f8[t, :, :])
                        nat_tiles[t] = nt
                        t2_tiles[t] = t2t
                    nt = nat_tiles[t]
                    t2t = t2_tiles[t]
                    c0n = i * 1024
                    c0t = i * T2W

                    # scores: 8 fp8 matmuls -> cols 0:64; cols 64:66 hold the
                    # softmax-denominator chain outputs (same PSUM bank).
                    sc_ps = ps_sc.tile([128, 66], F32, tag="sc")
                    md = t2t[:, c0t + 1024 : c0t + 1032]
                    for j in range(8):
                        nc.tensor.matmul(
                            sc_ps[:, j * 8 : (j + 1) * 8],
                            t2t[:, c0t + j * 128 : c0t + (j + 1) * 128],
                            md,
                            start=True,
                            stop=True,
                        )
                    e_sb = ep.tile([128, 64], BF16, tag="esc")
                    nc.scalar.activation(
                        out=e_sb, in_=sc_ps[:, 0:64], func=AF.Exp, scale=0.0625
                    )

                    # softmax denominator: s[nh] = sum_q e[q, nh]; den[h] = H4.T @ s
                    nc.tensor.matmul(
                        sc_ps[0:64, 64:65], e_sb, ones_col, start=True, stop=True
                    )
                    s_sb = ep.tile([64, 1], BF16, tag="ssb")
                    nc.scalar.activation(
                        out=s_sb, in_=sc_ps[0:64, 64:65], func=AF.Copy
                    )
                    nc.tensor.matmul(
                        sc_ps[0:4, 65:66], h4_sb, s_sb, start=True, stop=True
                    )
                    inv4 = ep.tile([4, 1], F32, tag="inv")
                    nc.vector.reciprocal(inv4, sc_ps[0:4, 65:66])

                    # u: two interleaved accumulation chains (even/odd chunks)
                    u_ps = ps_u.tile([4, 128], F32, tag="ub")
                    for nn_ in range(NCH):
                        half = nn_ % 2
                        nc.tensor.matmul(
                            u_ps[:, half * 64 : half * 64 + 64],
                            e_sb[:, 4 * nn_ : 4 * nn_ + 4],
                            _sap(nt, c0n + nn_, [[16, 64]]),
                            start=(nn_ == half),
                            stop=(nn_ >= NCH - 2),
                        )
                    u1_sb = ep.tile([4, 64], BF16, tag="u1")
                    nc.scalar.activation(
                        out=u1_sb, in_=u_ps[:, 0:64], func=AF.Copy, scale=inv4[:, :]
                    )
                    u_sb = ep.tile([4, 64], BF16, tag="usb")
                    nc.vector.scalar_tensor_tensor(
                        out=u_sb,
                        in0=u_ps[:, 64:128],
                        scalar=inv4[:, :],
                        in1=u1_sb,
                        op0=ALU.mult,
                        op1=ALU.add,
                    )
                    ut_ps = ps_u.tile([64, 4], BF16, tag="ub")
                    nc.tensor.transpose(ut_ps, u_sb, ident4)
                    nc.scalar.activation(
                        out=utg[:, bl * 4 : (bl + 1) * 4], in_=ut_ps, func=AF.Copy
                    )

                # ================= phase B: group value + address =================
                utg_v = utg[:].rearrange("p (b h) -> p h b", h=4)
                v_ps = ps_g.tile([64, G], F32, tag="g")
                for h in range(4):
                    nc.tensor.matmul(
                        v_ps,
                        wovT_sb[:, h * 64 : (h + 1) * 64],
                        utg_v[:, h : h + 1, :],
                        start=(h == 0),
                        stop=(h == 3),
                    )
                v65 = grp.tile([65, G], BF16, tag="v65")
                nc.scalar.activation(out=v65[0:64, :], in_=v_ps, func=AF.Copy)
                nc.vector.memset(v65[64:65, :], 1.0)

                h1_ps = ps_g.tile([128, G], F32, tag="g")
                nc.tensor.matmul(h1_ps, wa1T_sb, v65, start=True, stop=True)
                h1_sb = grp.tile([128, G], BF16, tag="h1")
                nc.vector.tensor_scalar_max(h1_sb, h1_ps, 0.0)

                wt_ps = ps_g.tile([16, 64], F32, tag="g")
                nc.tensor.matmul(wt_ps, v65, wwT_sb, start=True, stop=True)
                wt_sb = grp.tile([16, 64], BF16, tag="wt")
                nc.scalar.activation(out=wt_sb, in_=wt_ps, func=AF.Copy)

                # logitsT[q, k*16+bl]
                lg_ps = ps_g.tile([128, 16 * G], F32, tag="g")
                for k in range(16):
                    nc.tensor.matmul(
                        lg_ps[:, k * G : (k + 1) * G],
                        wa2T_sb[:, k * 128 : (k + 1) * 128],
                        h1_sb,
                        start=True,
                        stop=True,
                    )
                e2T = grp.tile([128, 16 * G], BF16, tag="e2T")
                nc.scalar.activation(out=e2T, in_=lg_ps, func=AF.Exp)

                # denominators: column-sum via ones matmul, then strided reduce
                ds_ps = ps_g.tile([1, 16 * G], F32, tag="g")
                nc.tensor.matmul(ds_ps, ones_col, e2T, start=True, stop=True)
                den_sb = grp.tile([1, G], F32, tag="den")
                nc.vector.reduce_sum(
                    out=den_sb,
                    in_=_sap(ds_ps, 0, [[1, G], [G, 16]]),
                    axis=mybir.AxisListType.X,
                )
                linv_sb = grp.tile([1, G], BF16, tag="linv")
                with nc.allow_low_precision(
                    reason="softmax 1/denom in bf16; scales a tiny update term"
                ):
                    nc.vector.reciprocal(linv_sb, den_sb)
                lb_ps = ps_g.tile([128, G], F32, tag="g")
                nc.tensor.matmul(lb_ps, ones_row, linv_sb, start=True, stop=True)

                # a2T[q, bl*16+k] = e2T[q, k*16+bl] * linv[bl]
                a2T = grp.tile([128, 16 * G], BF16, tag="a2T")
                nc.vector.tensor_tensor(
                    out=_sap(a2T, 0, [[G, 16], [1, 16]]),
                    in0=_sap(e2T, 0, [[1, 16], [G, 16]]),
                    in1=_sap(lb_ps, 0, [[1, 16], [0, 16]]),
                    op=ALU.mult,
                )

                # ============== phase C: per-block (4 batches) update ==============
                for tb in range(G // 4):
                    t = g * 4 + tb
                    bl0 = tb * 4
                    nt = nat_tiles.pop(t)
                    t2_tiles.pop(t, None)

                    wb_ps = ps_wb.tile([128, 256], F32, tag="wb")
                    for i in range(4):
                        nc.tensor.matmul(
                            wb_ps[:, i * 64 : (i + 1) * 64],
                            sel_sb[:, (bl0 + i) * 128 : (bl0 + i + 1) * 128],
                            wt_sb,
                            start=True,
                            stop=True,
                        )
                    wb_sb = wbp.tile([128, 256], BF16, tag="wbs")
                    nc.scalar.activation(out=wb_sb, in_=wb_ps, func=AF.Copy)

                    idn = [[1024, 4], [16, 64], [1, 16]]
                    d4 = dp.tile([128, 4096], BF16, tag="dt")
                    eng = nc.gpsimd if (t % 4) != 3 else nc.vector
                    eng.tensor_tensor(
                        out=_sap(d4, 0, idn),
                        in0=_sap(wb_sb, 0, [[64, 4], [1, 64], [0, 16]]),
                        in1=_sap(nt, 0, idn),
                        op=ALU.subtract,
                    )
                    p4 = pp.tile([128, 4096], BF16, tag="pt")
                    nc.vector.tensor_tensor(
                        out=_sap(p4, 0, idn),
                        in0=_sap(d4, 0, idn),
                        in1=_sap(a2T, bl0 * 16, [[16, 4], [0, 64], [1, 16]]),
                        op=ALU.mult,
                    )
                    ot = outp.tile([128, 4096], BF16, tag="ob")
                    nc.vector.scalar_tensor_tensor(
                        out=ot[:, :],
                        in0=nt[:, :],
                        scalar=1.0,
                        in1=p4[:, :],
                        op0=ALU.mult,
                        op1=ALU.add,
                    )
                    nc.sync.dma_start(out=out4[t, :, :], in_=ot)

    _split_dma_waits(nc)
    _CACHE["nc"] = nc
    return nc


def _split_dma_waits(nc):
    """walrus's DMA pseudo-instruction encodes at most one sem wait; move
    extra waits emitted by Tile onto a NoOp right before the DMA."""
    k = 0
    for f in nc.m.functions:
        for blk in f.blocks:
            insts = list(blk.instructions)
            new = []
            changed = False
            for inst in insts:
                si = inst.sync_info
                if si is not None and len(si.on_wait) > 1:
                    waits = list(si.on_wait)
                    for w in waits[:-1]:
                        nop = mybir.InstNoOp(name=f"WSPLIT-{k}", ins=[], outs=[])
                        k += 1
                        nop.engine = inst.engine
                        nop.sync_info = bass_rust.SyncInfo(
                            on_wait=[w], on_update=[]
                        )
                        new.append(nop)
                    inst.sync_info = bass_rust.SyncInfo(
                        on_wait=[waits[-1]], on_update=list(si.on_update)
                    )
                    changed = True
                new.append(inst)
            if changed:
                blk.instructions = new


def _host_prep(query, scratchpad, wq, wk, wv, bq, wo, w_a1, b_a1, w_a2,
               w_write, b_write):
    """Build per-core input maps (numpy, all host-side)."""
    bf = ml_dtypes.bfloat16
    f8 = ml_dtypes.float8_e4m3
    # query-side collapse: M[b,h,:] = 0.25 * Wk_h^T q_h, scaled x16 for fp8
    q = query[:, 0, :] @ wq.T + bq                         # [B, 64]
    M = 4.0 * np.einsum(
        "hjd,bhj->bhd", wk.reshape(H, HD, D), q.reshape(B, H, HD)
    )                                                      # [B, H, 64], 16*0.25
    mdup = np.zeros((B, 128, 8), np.float32)
    mt = M.transpose(0, 2, 1)                              # [B, 64, H]
    mdup[:, 0::2, 0:4] = mt                                # T2 partition = 2d+c
    mdup[:, 1::2, 4:8] = mt

    # t2 blocks: spT2 + md, fp8, 4 batches per row
    spT2 = np.ascontiguousarray(
        scratchpad.reshape(B, 8, 2, 128, 64)
        .transpose(0, 4, 2, 1, 3)     # [b, d, c, j, q] -> partition 2d+c
        .reshape(B, 128, 1024)
    )
    t2 = np.concatenate([spT2, mdup], axis=2).astype(f8)   # [B, 128, 1032]
    t2 = (
        t2.reshape(B // 4, 4, 128, T2W)
        .transpose(0, 2, 1, 3)
        .reshape(B // 4, 128, 4 * T2W)
    )

    # native blocks: (d,n) layout, bf16, 4 batches per row
    nat = (
        scratchpad.reshape(B, NCH, 128, 64)
        .transpose(0, 2, 3, 1)        # [b, q, d, n]
        .reshape(B, 128, 1024)
        .astype(bf)
    )
    nat = (
        nat.reshape(B // 4, 4, 128, 1024)
        .transpose(0, 2, 1, 3)
        .reshape(B // 4, 128, 4096)
    )

    wovT = np.concatenate(
        [(wo[:, h * HD : (h + 1) * HD] @ wv[h * HD : (h + 1) * HD, :]).T
         for h in range(H)],
        axis=1,
    )                                                      # [64, 256]
    wa1T65 = np.concatenate([w_a1.T, b_a1[None, :]], axis=0)   # [65, 128]
    wa2T = np.ascontiguousarray(w_a2.T)                    # [128, 2048]
    wwT65 = np.concatenate([w_write.T, b_write[None, :]], axis=0)  # [65, 64]
    sel = np.zeros((16, 2048), np.float32)
    for p in range(16):
        sel[p, p * 128 : (p + 1) * 128] = 1.0
    h4 = np.zeros((64, 4), np.float32)
    for p in range(64):
        h4[p, p % 4] = 1.0

    shared = {
        "wovT": wovT.astype(bf),
        "wa1T65": np.ascontiguousarray(wa1T65).astype(bf),
        "wa2T": wa2T.astype(bf),
        "wwT65": np.ascontiguousarray(wwT65).astype(bf),
        "sel": sel.astype(bf),
        "h4": h4.astype(bf),
    }
    in_maps = []
    for c in range(NCORES):
        lo, hi = c * NB4, (c + 1) * NB4
        in_maps.append(
            {"nat4": nat[lo:hi], "t2f8": t2[lo:hi], **shared}
        )
    return in_maps


def run(inputs, trace=False, **trace_kwargs):
    nc = _build()
    in_maps = _host_prep(
        np.asarray(inputs["query"], np.float32),
        np.asarray(inputs["scratchpad"], np.float32),
        np.asarray(inputs["wq"], np.float32),
        np.asarray(inputs["wk"], np.float32),
        np.asarray(inputs["wv"], np.float32),
        np.asarray(inputs["bq"], np.float32),
        np.asarray(inputs["wo"], np.float32),
        np.asarray(inputs["w_a1"], np.float32),
        np.asarray(inputs["b_a1"], np.float32),
        np.asarray(inputs["w_a2"], np.float32),
        np.asarray(inputs["w_write"], np.float32),
        np.asarray(inputs["b_write"], np.float32),
    )
    res = run_bass_kernel_spmd(
        nc, in_maps, core_ids=list(range(NCORES)), trace=trace, **trace_kwargs
    )
    outs = []
    for c in range(NCORES):
        o = np.asarray(res.results[c]["out4"]).astype(np.float32)
        outs.append(
            o.reshape(NB4, 128, 4, 64, 16)
            .transpose(0, 2, 4, 1, 3)     # [t, i, n, q, d]
            .reshape(BL, S, D)
        )
    full = np.concatenate(outs, axis=0)
    return full, res


def kernel(**inputs):
    full, _ = run(inputs, trace=False)
    return full
